# revision 7
# baseline (speedup 1.0000x reference)
"""Trainium2 Bass kernel: float32 -> 32-channel bit-plane encoding.

For input x [4096, 512] f32, produces out [4096, 512, 32] f32 where
out[b, f, 0] = (x[b,f] < 0) and out[b, f, 1+j] = bit (30-j) of
bitcast_int32(|x[b,f]|), MSB first.  Equivalently channels 0..31 are the
32 bits of bitcast_int32(x) MSB-first, except channel 0 uses the float
compare (x < 0) exactly as the reference does.

Sharded row-wise over 8 NeuronCores (512 rows each).  Per core:
  pass1 (VectorE):  and_t[p, f*32+k] = i[p,f] & mask[k]   (mask packed into
                    the input's last 32 columns so one DMA feeds both operands)
  pass2 (ScalarE):  out = Sign(and_t)  -- int32 -> f32, {0, 2^s} -> {0.0, 1.0}
                    (masks for k>=1 are positive; k=0 uses a bit30 placeholder)
  pass3 (VectorE):  out[:, k=0 plane] = (x < 0) via f32 is_lt on the bitcast
  DMA out via HWDGE (sync engine), 4MB contiguous row-block writes.
"""

import sys

if "/opt/trn_rl_repo" not in sys.path:
    sys.path.insert(0, "/opt/trn_rl_repo")

import numpy as np

import concourse.bass as bass
import concourse.mybir as mybir

P = 128          # SBUF partitions
F = 512          # features per row
K = 32           # output channels per feature
N_CORES = 8
ROWS_TOTAL = 4096
ROWS = ROWS_TOTAL // N_CORES   # rows per core
NRT = ROWS // P                # row tiles per core (4)
FCH = 256                      # feature chunk per tile
NFC = F // FCH                 # feature chunks (2)
NT = NRT * NFC                 # tiles per core (8)
XW = F + K                     # packed input width (x columns + mask columns)


def _masks_np() -> np.ndarray:
    # channel k>=1 tests bit (31-k); channel 0 is a bit30 placeholder that
    # pass3 overwrites with the float sign compare.
    vals = [1 << 30] + [1 << (31 - k) for k in range(1, K)]
    return np.array(vals, dtype=np.int64).astype(np.uint32).view(np.int32)


def build_nc() -> bass.Bass:
    nc = bass.Bass("TRN2", target_bir_lowering=False, debug=False)
    i32, f32 = mybir.dt.int32, mybir.dt.float32

    xm = nc.declare_dram_parameter("xm", [ROWS, XW], i32, isOutput=False)
    out = nc.declare_dram_parameter("out", [ROWS, F * K], f32, isOutput=True)
    xm_ap, out_ap = xm.ap(), out.ap()

    from contextlib import ExitStack
    with ExitStack() as ctx:
        xt = [ctx.enter_context(nc.sbuf_tensor(f"xt{b}", [P, XW], i32))
              for b in range(2)]
        at = [ctx.enter_context(nc.sbuf_tensor(f"at{b}", [P, FCH * K], i32))
              for b in range(2)]
        ot = [ctx.enter_context(nc.sbuf_tensor(f"ot{b}", [P, FCH * K], f32))
              for b in range(2)]

        # DMA-completion semaphores are per buffer-parity: concurrent DMAs on
        # one semaphore interleave their 16 per-engine increments, making a
        # ">= 16*k" wait ambiguous.
        in_sem = [ctx.enter_context(nc.semaphore(f"in_sem{b}")) for b in range(2)]
        od_sem = [ctx.enter_context(nc.semaphore(f"od_sem{b}")) for b in range(2)]
        tt_sem = ctx.enter_context(nc.semaphore("tt_sem"))
        act_sem = ctx.enter_context(nc.semaphore("act_sem"))
        p3_sem = ctx.enter_context(nc.semaphore("p3_sem"))

        ctx.enter_context(nc.Block())
        block = nc.cur_block

        @block.gpsimd
        def _(gp: bass.BassEngine):
            for rt in range(NRT):
                if rt >= 2:
                    # xt[rt%2] is free once pass3 of the last tile of
                    # row-block rt-2 has read it.
                    gp.wait_ge(p3_sem, (rt - 2) * NFC + NFC)
                gp.dma_start(
                    xt[rt % 2][:], xm_ap[rt * P:(rt + 1) * P, :]
                ).then_inc(in_sem[rt % 2], 16)

        def p3(vec: bass.BassEngine, t: int):
            rt, c = divmod(t, NFC)
            vec.wait_ge(act_sem, t + 1)
            xf = xt[rt % 2][:, c * FCH:(c + 1) * FCH].bitcast(mybir.dt.float32)
            sgn = ot[t % 2][:].rearrange("p (f k) -> p f k", k=K)[:, :, 0:1]
            vec.tensor_scalar(
                sgn, xf.unsqueeze(-1), 0.0, None, mybir.AluOpType.is_lt
            ).then_inc(p3_sem)

        @block.vector
        def _(vec: bass.BassEngine):
            for t in range(NT):
                rt, c = divmod(t, NFC)
                if c == 0:
                    vec.wait_ge(in_sem[rt % 2], 16 * (rt // 2 + 1))
                if t >= 2:
                    # at[t%2] is free once Sign(t-2) has read it.
                    vec.wait_ge(act_sem, t - 1)
                in0 = xt[rt % 2][:, c * FCH:(c + 1) * FCH].unsqueeze(-1) \
                    .broadcast_to([P, FCH, K])
                in1 = xt[rt % 2][:, F:F + K].unsqueeze(1) \
                    .broadcast_to([P, FCH, K])
                o3 = at[t % 2][:].rearrange("p (f k) -> p f k", k=K)
                vec.tensor_tensor(
                    o3, in0, in1, mybir.AluOpType.bitwise_and
                ).then_inc(tt_sem)
                if t >= 1:
                    p3(vec, t - 1)
            p3(vec, NT - 1)

        @block.scalar
        def _(sc: bass.BassEngine):
            for t in range(NT):
                sc.wait_ge(tt_sem, t + 1)
                if t >= 2:
                    # ot[t%2] is free once out-DMA(t-2) has drained it.
                    sc.wait_ge(od_sem[t % 2], 16 * (t // 2))
                sc.activation(
                    ot[t % 2][:], at[t % 2][:], mybir.ActivationFunctionType.Sign
                ).then_inc(act_sem)

        @block.sync
        def _(sp: bass.BassEngine):
            for t in range(NT):
                rt, c = divmod(t, NFC)
                sp.wait_ge(p3_sem, t + 1)
                sp.dma_start(
                    out_ap[rt * P:(rt + 1) * P, c * FCH * K:(c + 1) * FCH * K],
                    ot[t % 2][:],
                ).then_inc(od_sem[t % 2], 16)

    return nc


_NC_CACHE = None


def _get_nc():
    global _NC_CACHE
    if _NC_CACHE is None:
        _NC_CACHE = build_nc()
    return _NC_CACHE


def pack_shard(x_shard: np.ndarray) -> np.ndarray:
    """[ROWS, F] f32 -> [ROWS, F+K] int32 with mask columns appended."""
    xi = np.ascontiguousarray(x_shard).view(np.int32)
    m = np.broadcast_to(_masks_np(), (x_shard.shape[0], K))
    return np.ascontiguousarray(np.concatenate([xi, m], axis=1))


def kernel(x: np.ndarray) -> np.ndarray:
    from concourse.bass_utils import run_bass_kernel_spmd

    x = np.asarray(x, dtype=np.float32)
    assert x.shape == (ROWS_TOTAL, F), x.shape
    nc = _get_nc()
    in_maps = [
        {"xm": pack_shard(x[i * ROWS:(i + 1) * ROWS])} for i in range(N_CORES)
    ]
    res = run_bass_kernel_spmd(nc, in_maps, list(range(N_CORES)))
    parts = [res.results[i]["out"].reshape(ROWS, F, K) for i in range(N_CORES)]
    return np.concatenate(parts, axis=0)


# revision 12
# speedup vs baseline: 1.1157x; 1.1157x over previous
"""Trainium2 Bass kernel: float32 -> 32-channel bit-plane encoding.

For input x [4096, 512] f32, produces out [4096, 512, 32] f32 where
out[b, f, 0] = (x[b,f] < 0) and out[b, f, 1+j] = bit (30-j) of
bitcast_int32(|x[b,f]|), MSB first.

Sharded row-wise over 8 NeuronCores (512 rows each).  Per core:
  pass1 (VectorE):  and_t[p, f, k] = i[p,f] & mask[k]  (masks packed into the
                    input's last columns so one DMA feeds both operands)
  pass2 (ScalarE):  Sign(and_t) -> f32 0/1 bit channels
  pass3 (VectorE):  channel-0 plane = (x < 0) via f32 is_lt on the bitcast
  out-DMA via HWDGE (sync engine).

The out-DMA stream is the bottleneck (~32MB/core at ~400GB/s); the schedule
uses small leading chunks so the DMA stream starts as early as possible and
then stays continuously busy.
"""

import sys

if "/opt/trn_rl_repo" not in sys.path:
    sys.path.insert(0, "/opt/trn_rl_repo")

import numpy as np

import concourse.bass as bass
import concourse.mybir as mybir

P = 128          # SBUF partitions
F = 512          # features per row
K = 32           # output channels per feature
KB = K - 1       # bit channels 1..31
N_CORES = 8
ROWS_TOTAL = 4096
ROWS = ROWS_TOTAL // N_CORES   # rows per core
NRT = ROWS // P                # row tiles per core (4)
XW = F + K                     # packed input width (x columns + 32 mask cols)
FCH_MAX = 256

# Feature chunks per row block: small leading chunks collapse the pipeline
# ramp (first out-DMA starts after ~2us of compute instead of ~16us).
CHUNKS_RB0_SMALL = [32, 32, 64, 128, 256]
CHUNKS_RB = [256, 256]

NBUF_AT = 2
NBUF_OT = 3


def _masks_np() -> np.ndarray:
    # mask column k holds the mask for output channel k; column 0 is a bit30
    # placeholder (its output is overwritten by the pass3 sign compare).
    vals = [1 << 30] + [1 << (31 - k) for k in range(1, K)]
    return np.array(vals, dtype=np.int64).astype(np.uint32).view(np.int32)


def _tiles(small_chunks=True):
    """Yield (t, rt, c_off, c_len) in schedule order."""
    t = 0
    for rt in range(NRT):
        off = 0
        chunks = CHUNKS_RB0_SMALL if (rt == 0 and small_chunks) else CHUNKS_RB
        for c_len in chunks:
            yield t, rt, off, c_len
            off += c_len
            t += 1
        assert off == F


def build_nc(strided_sign=True, act_dma=True, warm_act=True,
             small_chunks=True) -> bass.Bass:
    nc = bass.Bass("TRN2", target_bir_lowering=False, debug=False)
    i32, f32 = mybir.dt.int32, mybir.dt.float32

    xm = nc.declare_dram_parameter("xm", [ROWS, XW], i32, isOutput=False)
    out = nc.declare_dram_parameter("out", [ROWS, F * K], f32, isOutput=True)
    xm_ap, out_ap = xm.ap(), out.ap()

    tiles = list(_tiles(small_chunks))
    kch = KB if strided_sign else K   # and-tile channels per feature

    from contextlib import ExitStack
    with ExitStack() as ctx:
        # one xt buffer per row block: no reuse, all four in-DMAs prefetch
        xt = [ctx.enter_context(nc.sbuf_tensor(f"xt{b}", [P, XW], i32))
              for b in range(NRT)]
        at = [ctx.enter_context(nc.sbuf_tensor(f"at{b}", [P, FCH_MAX * kch], i32))
              for b in range(NBUF_AT)]
        ot = [ctx.enter_context(nc.sbuf_tensor(f"ot{b}", [P, FCH_MAX * K], f32))
              for b in range(NBUF_OT)]
        warm = ctx.enter_context(nc.sbuf_tensor("warm", [P, 1], f32))

        in_sem = [ctx.enter_context(nc.semaphore(f"in_sem{b}")) for b in range(NRT)]
        od_sem = [ctx.enter_context(nc.semaphore(f"od_sem{b}"))
                  for b in range(NBUF_OT)]
        tt_sem = ctx.enter_context(nc.semaphore("tt_sem"))
        act_sem = ctx.enter_context(nc.semaphore("act_sem"))
        p3_sem = ctx.enter_context(nc.semaphore("p3_sem"))

        ctx.enter_context(nc.Block())
        block = nc.cur_block

        def p3(vec, t, rt, c_off, c_len):
            """channel-0 plane = (x < 0); on DVE."""
            if t >= NBUF_OT:
                vec.wait_ge(od_sem[t % NBUF_OT], 16 * (t // NBUF_OT))
            xf = xt[rt][:, c_off:c_off + c_len].bitcast(f32)
            sgn = ot[t % NBUF_OT][:, 0:c_len * K] \
                .rearrange("p (f k) -> p f k", k=K)[:, :, 0:1]
            vec.tensor_scalar(
                sgn, xf.unsqueeze(-1), 0.0, None, mybir.AluOpType.is_lt
            ).then_inc(p3_sem)

        @block.vector
        def _(vec: bass.BassEngine):
            seen_rb = -1
            for t, rt, c_off, c_len in tiles:
                if rt != seen_rb:
                    vec.wait_ge(in_sem[rt], 16)
                    seen_rb = rt
                if t >= NBUF_AT:
                    # at[t%NBUF_AT] is free once Sign(t-NBUF_AT) has read it
                    vec.wait_ge(act_sem, t - NBUF_AT + 1)
                moff = F if not strided_sign else F + 1
                in0 = xt[rt][:, c_off:c_off + c_len].unsqueeze(-1) \
                    .broadcast_to([P, c_len, kch])
                in1 = xt[rt][:, moff:moff + kch].unsqueeze(1) \
                    .broadcast_to([P, c_len, kch])
                o3 = at[t % NBUF_AT][:, 0:c_len * kch] \
                    .rearrange("p (f k) -> p f k", k=kch)
                vec.tensor_tensor(
                    o3, in0, in1, mybir.AluOpType.bitwise_and
                ).then_inc(tt_sem)
                if strided_sign:
                    # ot channel 0 is written only by pass3: do it right away
                    p3(vec, t, rt, c_off, c_len)
                else:
                    # ot fully written by Sign; pass3 overwrites channel 0
                    # afterwards -> run pass3 one tile behind
                    if t >= 1:
                        pt, prt, pco, pcl = tiles[t - 1]
                        vec.wait_ge(act_sem, t)
                        p3(vec, pt, prt, pco, pcl)
            if not strided_sign:
                pt, prt, pco, pcl = tiles[-1]
                vec.wait_ge(act_sem, len(tiles))
                p3(vec, pt, prt, pco, pcl)

        @block.scalar
        def _(sc: bass.BassEngine):
            if act_dma:
                for rt in range(NRT):
                    sc.dma_start(
                        xt[rt][:], xm_ap[rt * P:(rt + 1) * P, :]
                    ).then_inc(in_sem[rt], 16)
            if warm_act:
                # scale=0 -> input is not read (safe on uninitialized SBUF)
                sc.activation(warm[:], warm[:],
                              mybir.ActivationFunctionType.Sign, scale=0.0)
            for t, rt, c_off, c_len in tiles:
                sc.wait_ge(tt_sem, t + 1)
                if t >= NBUF_OT:
                    # ot[t%NBUF_OT] is free once out-DMA(t-NBUF_OT) drained it
                    sc.wait_ge(od_sem[t % NBUF_OT], 16 * (t // NBUF_OT))
                a_in = at[t % NBUF_AT][:, 0:c_len * kch]
                if strided_sign:
                    o_out = ot[t % NBUF_OT][:, 0:c_len * K] \
                        .rearrange("p (f k) -> p f k", k=K)[:, :, 1:K]
                else:
                    o_out = ot[t % NBUF_OT][:, 0:c_len * K]
                sc.activation(
                    o_out, a_in, mybir.ActivationFunctionType.Sign
                ).then_inc(act_sem)

        if not act_dma:
            @block.gpsimd
            def _(gp: bass.BassEngine):
                for rt in range(NRT):
                    gp.dma_start(
                        xt[rt][:], xm_ap[rt * P:(rt + 1) * P, :]
                    ).then_inc(in_sem[rt], 16)

        @block.sync
        def _(sp: bass.BassEngine):
            for t, rt, c_off, c_len in tiles:
                sp.wait_ge(act_sem, t + 1)
                sp.wait_ge(p3_sem, t + 1)
                sp.dma_start(
                    out_ap[rt * P:(rt + 1) * P,
                           c_off * K:(c_off + c_len) * K],
                    ot[t % NBUF_OT][:, 0:c_len * K],
                ).then_inc(od_sem[t % NBUF_OT], 16)

    return nc


_NC_CACHE = None


def _get_nc():
    global _NC_CACHE
    if _NC_CACHE is None:
        _NC_CACHE = build_nc(strided_sign=False, act_dma=False)
    return _NC_CACHE


def pack_shard(x_shard: np.ndarray) -> np.ndarray:
    """[ROWS, F] f32 -> [ROWS, F+K] int32 with mask columns appended."""
    xi = np.ascontiguousarray(x_shard).view(np.int32)
    m = np.broadcast_to(_masks_np(), (x_shard.shape[0], K))
    return np.ascontiguousarray(np.concatenate([xi, m], axis=1))


def kernel(x: np.ndarray) -> np.ndarray:
    from concourse.bass_utils import run_bass_kernel_spmd

    x = np.asarray(x, dtype=np.float32)
    assert x.shape == (ROWS_TOTAL, F), x.shape
    nc = _get_nc()
    in_maps = [
        {"xm": pack_shard(x[i * ROWS:(i + 1) * ROWS])} for i in range(N_CORES)
    ]
    res = run_bass_kernel_spmd(nc, in_maps, list(range(N_CORES)))
    parts = [res.results[i]["out"].reshape(ROWS, F, K) for i in range(N_CORES)]
    return np.concatenate(parts, axis=0)
